# revision 38
# baseline (speedup 1.0000x reference)
"""Multi-head causal attention (B=2, S=2048, D=2048, H=16) on 8 trn2 cores.

Sharding: tensor-parallel over heads. Core c owns heads 2c, 2c+1 (256
features of q/k/v). Each core computes its heads' QKV projections (+RoPE),
causal attention, and a partial output through its slice of wo. The 8
partial outputs are summed on the host (the "all-reduce").

All matmul operands are bf16 (fp32 PSUM accumulate); IO tensors are bf16
(halves HBM traffic, enables FWL weight loads). f32r and bf16 both stream
1 cycle/row on the PE, so precision is the only trade (measured ~4e-3 rel).

Per-core layouts:
  qT, kT: [hd=128 partitions, head, token] bf16. Head dims permuted (evens
          then odds) via host-permuted wq/wk rows so RoPE pairs sit in
          partition halves.
  v:      natural [token, feature] bf16, computed directly with x chunks as
          the stationary operand (no PE transposes).

PSUM is managed as 2-bank pair tiles [128, 2, 512] (4 pairs + attn accum +
wo out = 8 banks). Pairing lets one ACT instruction cover 1024 columns,
halving the per-instruction overhead (352 cycles) on the exp-critical
attention path:
  scoresT pair = 2 matmuls (lhsT=kT chunk, rhs=qT tile) -> [kt 128, 2, 512]
  probsT  pair = one 1024-wide exp on ACT -> bf16 SBUF (scores O(1), no max)
  denom   = running bf16 chunk-sum on DVE, ones-matmul partition broadcast,
            reciprocal_approx_fast on DVE.
  attnT accum = matmul(lhsT=v chunk, rhs=probsT half) -> [hd, q] PSUM,
            normalized into aT bf16 at eviction.
  out rows = matmul(lhsT=aT t-sub, rhs=woT) -> [t 128, j 512] PSUM, staged
            into a [128, 4, 2048] bf16 tile, one 2MB DMA per (b, qtile)
            (final tile: per-512-col 128KB stores to shrink the kernel tail).

QKV runs as three paired passes per token tile (q01, k01, v) through the
same pair pool. RoPE per paired eviction qp [128, 2, 512] (top partition
rows even dims xr, bottom odd xi):
  qraw = copy(qp)->bf16          (one 1024-wide ACT copy; frees the banks)
  p1   = qraw*[c;c]              (DVE)
  t2   = swap-halves mul: t2[0:64]=qraw[64:128]*s, t2[64:128]=qraw[0:64]*-s
         via partition-offset DVE reads (no ACT half-copies)
  qT   = p1 - t2                 (DVE)

wo jobs are queued per attention tile and paced into the NEXT attention
tile's chunk loop only (QKV phases are already PE-dense; draining there
just starves the exp-bound attention stretches of PE fill work).

Startup: the first x tile is loaded in 2-chunk slices and warm-up matmuls
on a zeroed SBUF tile are interleaved into the first q pass so the PE
never idles >1us waiting on HBM (keeps HAM at K=8/8 from the start).
"""

import math

import numpy as np

B = 2
S = 2048
D = 2048
H = 16
HD = 128
NCORES = 8
FPC = D // NCORES          # 256 features (2 heads) per core
P = 128
ND = D // P                # 16 contraction chunks
TT = 512                   # token tile (matmul free dim)
NTT = S // TT              # 4 token tiles per batch
NKT = S // P               # 16 key chunks per batch
SCALE = 1.0 / math.sqrt(HD)

_CACHE = {}


def _build_nc():
    import concourse.bass as bass  # noqa: F401
    from concourse import bacc
    import concourse.bass_isa as bass_isa
    import concourse.mybir as mybir
    import concourse.tile as tile

    f32 = mybir.dt.float32
    bf16 = mybir.dt.bfloat16
    MUL = mybir.AluOpType.mult
    SUB = mybir.AluOpType.subtract
    ADD = mybir.AluOpType.add
    EXP = mybir.ActivationFunctionType.Exp

    nc = bacc.Bacc(None, target_bir_lowering=False)

    # All inputs are host-pre-tiled to [128 partitions, ...contiguous] so
    # every DMA is 128 large contiguous descriptors (line-rate), not
    # thousands of 512B strided ones.
    NGT = B * NTT  # 8 global token tiles
    xTb = nc.dram_tensor("xTb", [P, NGT, ND, TT], bf16, kind="ExternalInput")
    wqT = nc.dram_tensor("wqT", [P, ND, FPC], bf16, kind="ExternalInput")
    wkT = nc.dram_tensor("wkT", [P, ND, FPC], bf16, kind="ExternalInput")
    wvT = nc.dram_tensor("wvT", [P, ND, FPC], bf16, kind="ExternalInput")
    woT = nc.dram_tensor("woT", [P, 2, D], bf16, kind="ExternalInput")
    cosS = nc.dram_tensor("cosS", [P, 2, S], bf16, kind="ExternalInput")
    sinS = nc.dram_tensor("sinS", [P, 2, S], bf16, kind="ExternalInput")
    masks = nc.dram_tensor("masks", [P, 2 * P], bf16, kind="ExternalInput")
    onesd = nc.dram_tensor("onesd", [P, P], bf16, kind="ExternalInput")
    outp = nc.dram_tensor("outp", [B * S, D], bf16, kind="ExternalOutput")

    outr = outp.rearrange("(r p) d -> p r d", p=P)     # [128, 32, D]

    with tile.TileContext(nc) as tc:
        with (
            tc.tile_pool(name="res", bufs=1) as res,
            tc.tile_pool(name="xp", bufs=2) as xp,
            tc.tile_pool(name="ropep", bufs=2) as ropep,
            tc.tile_pool(name="probsp", bufs=4) as probsp,
            tc.tile_pool(name="accp", bufs=2) as accp,
            tc.tile_pool(name="dp", bufs=2) as dp,
            tc.tile_pool(name="aTp", bufs=2) as aTp,
            tc.tile_pool(name="ostp", bufs=2) as ostp,
            # 2-bank pair tiles: QKV passes, score pairs, and denominator
            # tiles share one pool; + attn accum + wo out = 8 banks total.
            tc.tile_pool(name="ps_qs", bufs=2, space="PSUM") as ps_qs,
            tc.tile_pool(name="ps_a", bufs=2, space="PSUM") as ps_a,
            tc.tile_pool(name="ps_o", bufs=2, space="PSUM") as ps_o,
        ):
            wq_sb = res.tile([P, ND, FPC], bf16)
            wk_sb = res.tile([P, ND, FPC], bf16)
            wv_sb = res.tile([P, ND, FPC], bf16)
            wo_sb = res.tile([P, 2, D], bf16)
            cos_sb = res.tile([P, 2, S], bf16)
            sin_sb = res.tile([P, 2, S], bf16)
            mask_sb = res.tile([P, 2 * P], bf16)
            ones_sb = res.tile([P, P], bf16)
            qT_sb = res.tile([P, B, 2, S], bf16)
            kT_sb = res.tile([P, B, 2, S], bf16)
            v_sb = res.tile([P, B, NKT, FPC], bf16)

            # weight/constant loads all on the scalar HWDGE queue (a third
            # queue steals early HBM bandwidth from the critical wq/x
            # streams), ordered by first-use time. wq/wk stream in 128KB
            # 2-chunk slices so arrival tracks the first passes' chunk-wise
            # consumption (coarse slices stall the PE in bursts).
            for g in range(8):
                nc.scalar.dma_start(
                    out=wq_sb[:, 2 * g:2 * g + 2, :],
                    in_=wqT[:, 2 * g:2 * g + 2, :])
            for g in range(8):
                nc.scalar.dma_start(
                    out=wk_sb[:, 2 * g:2 * g + 2, :],
                    in_=wkT[:, 2 * g:2 * g + 2, :])
            nc.scalar.dma_start(out=cos_sb[:], in_=cosS[:])
            nc.scalar.dma_start(out=sin_sb[:], in_=sinS[:])
            for g in range(4):
                nc.scalar.dma_start(
                    out=wv_sb[:, 4 * g:4 * g + 4, :],
                    in_=wvT[:, 4 * g:4 * g + 4, :])
            nc.scalar.dma_start(out=mask_sb[:], in_=masks[:])
            nc.scalar.dma_start(out=ones_sb[:], in_=onesd[:])
            nc.scalar.dma_start(out=wo_sb[:], in_=woT[:])

            # HAM warm-up matmuls on zeroed SBUF: issued upfront AND
            # interleaved into the first q pass, so the PE is never idle
            # long enough (>3.4us) to re-throttle to K=4/8 while the
            # initial weight/x DMAs stream in.
            warm_sb = res.tile([P, TT], bf16)
            nc.vector.memset(warm_sb[:], 0)

            def warm(n):
                for _ in range(n):
                    w_ps = ps_o.tile([P, TT], f32, name="ops")
                    nc.tensor.matmul(
                        w_ps[:], warm_sb[:, 0:P], warm_sb[:],
                        start=True, stop=True)

            warm(10)

            wo_jobs = []
            ost_state = {}  # id(ost) -> [count, b, qt, fine]
            _alt = [0]

            def emit_wo_group(b, qt, aT, ts, jc, ost):
                o_ps = ps_o.tile([P, TT], f32, name="ops")
                for h in range(2):
                    nc.tensor.matmul(
                        o_ps[:],
                        aT[:, h, ts * P:(ts + 1) * P],
                        wo_sb[:, h, jc * TT:(jc + 1) * TT],
                        start=(h == 0), stop=(h == 1),
                    )
                # alternate eviction engines so consecutive groups never
                # queue behind one engine's FIFO
                _alt[0] ^= 1
                cp = nc.scalar.copy if _alt[0] else nc.vector.tensor_copy
                cp(ost[:, ts, jc * TT:(jc + 1) * TT], o_ps[:])
                st = ost_state[id(ost)]
                st[0] += 1
                r0 = (b * S + qt * TT) // P + ts
                # stores ride the sync queue, which carries nothing else
                # (x prefetch is on gpsimd): a store's completion wait can
                # only ever block later stores, never compute engines
                if st[3]:
                    # final tile: store each 128KB column group as soon as
                    # it lands (shrinks the kernel tail to one small DMA)
                    nc.sync.dma_start(
                        out=outr[:, r0:r0 + 1, jc * TT:(jc + 1) * TT],
                        in_=ost[:, ts:ts + 1, jc * TT:(jc + 1) * TT])
                elif st[0] % 4 == 0:
                    # jobs run ts-major: store each 512-token-row slab as
                    # soon as its 4 column groups land
                    tsd = st[0] // 4 - 1
                    r0 = (b * S + qt * TT) // P + tsd
                    nc.sync.dma_start(
                        out=outr[:, r0:r0 + 1, :], in_=ost[:, tsd:tsd + 1, :])

            def pop_wo(n=1):
                for _ in range(n):
                    if wo_jobs:
                        job = wo_jobs.pop(0)
                        if callable(job):
                            job()
                        else:
                            emit_wo_group(*job)

            def rope_finish(qraw, dst, tsl):
                p1 = ropep.tile([P, 2, TT], bf16, name="p1")
                nc.vector.tensor_tensor(
                    out=p1[:], in0=qraw[:], in1=cos_sb[:, :, tsl], op=MUL)
                # partition swap must go through ACT (DVE lanes are
                # partition-aligned; cross-partition writes fault on HW)
                qsw = ropep.tile([P, 2, TT], bf16, name="qsw")
                nc.scalar.copy(qsw[0:64, :, :], qraw[64:128, :, :])
                nc.scalar.copy(qsw[64:128, :, :], qraw[0:64, :, :])
                t2 = ropep.tile([P, 2, TT], bf16, name="t2")
                nc.vector.tensor_tensor(
                    out=t2[:], in0=qsw[:], in1=sin_sb[:, :, tsl], op=MUL)
                nc.vector.tensor_tensor(
                    out=dst[:, :, tsl], in0=p1[:], in1=t2[:], op=SUB)

            def rope_evict_pair(ps, dst, tsl, defer=False):
                # ps [P, 2, TT] f32 PSUM -> dst[:, :, tsl] with RoPE.
                # Top partition half = even dims (xr), bottom = odd (xi):
                #   out_top = xr*c - xi*s ; out_bot = xi*c + xr*s
                # via dst = qraw*[c;c] - swap(qraw)*[s;-s].
                qraw = ropep.tile([P, 2, TT], bf16, name="qraw")
                nc.scalar.copy(qraw[:], ps[:])
                if defer:
                    # tile tt=3's rotation output isn't consumed until
                    # attention qt=3: pace its ACT-heavy swap into the
                    # qt0 chunk loop instead of bursting it ahead of
                    # qt0's exps on the ACT FIFO
                    wo_jobs.append(lambda: rope_finish(qraw, dst, tsl))
                else:
                    rope_finish(qraw, dst, tsl)

            # x-tile prefetch: DMAs ride the sync HWDGE FIFO, emitted one
            # step ahead of the consumer. The first tile is loaded in
            # 2-chunk slices so the first q-pass progresses with arrival.
            xt_tiles = {}

            def prefetch_x(b, tt):
                gt = b * NTT + tt
                xt = xp.tile([P, ND, TT], bf16, name="xt")
                if b == 0 and tt == 0:
                    for g in range(8):
                        nc.sync.dma_start(
                            out=xt[:, 2 * g:2 * g + 2, :],
                            in_=xTb[:, gt, 2 * g:2 * g + 2, :])
                else:
                    nc.sync.dma_start(out=xt[:], in_=xTb[:, gt, :, :])
                xt_tiles[(b, tt)] = xt

            def emit_v_pass(b, tt, vp, xt):
                ps = ps_qs.tile([P, 2, TT], f32, name="qs")
                for i in range(2):
                    sub = 2 * vp + i
                    for d in range(ND):
                        nc.tensor.matmul(
                            ps[:, i, 0:FPC],
                            xt[:, d, sub * P:(sub + 1) * P],
                            wv_sb[:, d, :],
                            start=(d == 0), stop=(d == ND - 1),
                        )
                nc.any.tensor_copy(
                    v_sb[:, b, tt * 4 + 2 * vp:tt * 4 + 2 * vp + 2, :],
                    ps[:, :, 0:FPC])

            def emit_qkv(b, tt):
                tsl = slice(tt * TT, (tt + 1) * TT)
                nxt = (b, tt + 1) if tt + 1 < NTT else (b + 1, 0)
                first = (b == 0 and tt == 0)
                if nxt[0] < B and not first:
                    prefetch_x(*nxt)
                xt = xt_tiles.pop((b, tt))

                # q/k passes: one 2-bank pair per projection
                for w_sb, dst in ((wq_sb, qT_sb), (wk_sb, kT_sb)):
                    ps = ps_qs.tile([P, 2, TT], f32, name="qs")
                    for fc in range(2):
                        for d in range(ND):
                            nc.tensor.matmul(
                                ps[:, fc, :],
                                w_sb[:, d, fc * P:(fc + 1) * P],
                                xt[:, d, :],
                                start=(d == 0), stop=(d == ND - 1),
                            )
                            # bridge the initial HBM stream with warm-ups:
                            # generous fill — an early idle re-throttles
                            # HAM to half clock and it then can't recover
                            if first and d % 2 == 1:
                                if w_sb is wq_sb:
                                    warm(2 if fc == 0 else 1)
                                elif fc == 0:
                                    warm(1)
                    rope_evict_pair(ps, dst[:, b], tsl, defer=(tt == NTT - 1))
                if first:
                    prefetch_x(*nxt)
                if tt == NTT - 1:
                    # defer the last tile's v passes into the next
                    # attention tile (qt=0 has no wo fill work otherwise)
                    wo_jobs.append(
                        lambda: emit_v_pass(b, tt, 0, xt))
                    wo_jobs.append(
                        lambda: emit_v_pass(b, tt, 1, xt))
                else:
                    for vp in range(2):
                        emit_v_pass(b, tt, vp, xt)

            def emit_attention(b, qt):
                npair = 2 * qt           # full sub-diagonal chunk pairs
                aT = aTp.tile([P, 2, TT], bf16, name="aT")
                fine = (b == B - 1 and qt == NTT - 1)
                ost = ostp.tile([P, 4, D], bf16, name="ost")
                ost_state[id(ost)] = [0, b, qt, fine]
                # spread pending fill jobs over this tile's iterations
                niter = 2 * (npair + 4)
                wo_quota = len(wo_jobs)
                emitted = [0]
                it = [0]

                def pace():
                    it[0] += 1
                    want = wo_quota * it[0] // niter
                    while emitted[0] < want and wo_jobs:
                        job = wo_jobs.pop(0)
                        if callable(job):
                            job()
                        else:
                            emit_wo_group(*job)
                        emitted[0] += 1

                qsl = slice(qt * TT, (qt + 1) * TT)
                # previous head's denominator chain, deferred so it is
                # emitted AFTER the next head's first scores group: the
                # chain (last acc add -> ones-mm -> reciprocal -> aT mul)
                # then overlaps the next head's chunk stream instead of
                # bubbling the PE at the head boundary.
                chain = [None]

                def emit_chain():
                    if chain[0] is not None:
                        chain[0]()
                        chain[0] = None

                for h in range(2):
                    acc = accp.tile([P, TT], bf16, name="acc")
                    a_ps = ps_a.tile([P, TT], f32, name="aps")
                    # 1-deep software pipeline: the attnV matmuls for a
                    # chunk (which wait on its exp) are emitted AFTER the
                    # next chunk's scores matmuls, so a stalled attnV at
                    # the PE FIFO head never starves the scores stream.
                    pend = []  # [(pr_view, kt, csl), ...] awaiting attnV

                    def flush_av(last=False):
                        for n, (pv, kt, csl) in enumerate(pend):
                            nc.tensor.matmul(
                                a_ps[:, csl],
                                v_sb[:, b, kt, h * P:(h + 1) * P],
                                pv,
                                start=(kt == 0),
                                stop=(last and n == len(pend) - 1),
                            )
                        pend.clear()

                    for i in range(npair):
                        s_ps = ps_qs.tile([P, 2, TT], f32, name="qs")
                        for j in range(2):
                            kt = 2 * i + j
                            nc.tensor.matmul(
                                s_ps[:, j, :],
                                kT_sb[:, b, h, kt * P:(kt + 1) * P],
                                qT_sb[:, b, h, qsl],
                                start=True, stop=True,
                            )
                        # fill BEFORE the pending attnV: the fill has no
                        # deps, while attnV waits on its exp — this orders
                        # the PE FIFO [scores, fill, attnV] so the exp
                        # latency is fully covered
                        pace()
                        flush_av()
                        if i == 0:
                            emit_chain()
                        pr = probsp.tile([P, 2, TT], bf16, name="probs")
                        nc.scalar.activation(
                            pr[:], s_ps[:], EXP, scale=SCALE)
                        if i == 0:
                            nc.vector.tensor_tensor(
                                out=acc[:], in0=pr[:, 0, :],
                                in1=pr[:, 1, :], op=ADD)
                        else:
                            nc.vector.tensor_tensor(
                                out=acc[:], in0=acc[:],
                                in1=pr[:, 0, :], op=ADD)
                            nc.vector.tensor_tensor(
                                out=acc[:], in0=acc[:],
                                in1=pr[:, 1, :], op=ADD)
                        for j in range(2):
                            kt = 2 * i + j
                            pend.append((pr[:, j, :], kt, slice(0, TT)))
                    # 4 diagonal chunks, two per pair tile
                    for o in range(4):
                        kt = 4 * qt + o
                        c0 = o * P
                        csl = slice(c0, TT)
                        if o % 2 == 0:
                            d_tile = ps_qs.tile([P, 2, TT], f32, name="qs")
                            pr_d = probsp.tile([P, 2, TT], bf16, name="probs")
                        half = o % 2
                        nc.tensor.matmul(
                            d_tile[:, half, csl],
                            kT_sb[:, b, h, kt * P:(kt + 1) * P],
                            qT_sb[:, b, h, qt * TT + c0:(qt + 1) * TT],
                            start=True, stop=True,
                        )
                        pace()
                        flush_av()
                        if o == 0:
                            emit_chain()
                        nc.scalar.activation(
                            pr_d[:, half, csl], d_tile[:, half, csl],
                            EXP, scale=SCALE)
                        nc.vector.tensor_tensor(
                            out=pr_d[:, half, c0:c0 + P],
                            in0=pr_d[:, half, c0:c0 + P],
                            in1=mask_sb[:, P:2 * P], op=MUL)
                        if kt == 0:
                            nc.vector.tensor_copy(acc[:], pr_d[:, half, :])
                        else:
                            nc.vector.tensor_tensor(
                                out=acc[:, csl], in0=acc[:, csl],
                                in1=pr_d[:, half, csl], op=ADD)
                        pend.append((pr_d[:, half, csl], kt, csl))
                    flush_av(last=True)

                    # softmax denominator: ones-matmul broadcasts the
                    # partition-sum of acc to all 128 partitions in one
                    # 512-cycle PE op; ~1.5us chain latency to aT. Lives in
                    # the wo pool so the scores pair rotation never WARs
                    # against the reciprocal's read.
                    def mk_chain(acc=acc, a_ps=a_ps, h=h):
                        def c():
                            d_ps = ps_o.tile([P, TT], f32, name="ops")
                            nc.tensor.matmul(
                                d_ps[:], ones_sb[:], acc[:],
                                start=True, stop=True)
                            rb = dp.tile([P, TT], f32, name="rb")
                            nc.vector.reciprocal_approx_fast(rb[:], d_ps[:])
                            nc.vector.tensor_tensor(
                                out=aT[:, h, :], in0=a_ps[:], in1=rb[:],
                                op=MUL)
                        return c

                    chain[0] = mk_chain()
                emit_chain()
                for ts in range(4):
                    for jc in range(D // TT):
                        wo_jobs.append((b, qt, aT, ts, jc, ost))

            # schedule: all QKV tiles of a batch, then all its attention
            # tiles. wo jobs are paced ONLY into attention tiles (QKV is
            # already PE-dense); each batch's last-tile jobs carry across
            # the next batch's QKV phase into its first attention tile.
            prefetch_x(0, 0)
            for b in range(B):
                for tt in range(NTT):
                    emit_qkv(b, tt)
                for qt in range(NTT):
                    emit_attention(b, qt)
            pop_wo(len(wo_jobs))
    nc.compile()
    return nc


def _host_prep(x, wq, wk, wv, wo):
    import ml_dtypes

    bf = ml_dtypes.bfloat16
    x = np.asarray(x, dtype=np.float32)
    wq = np.asarray(wq, dtype=np.float32)
    wk = np.asarray(wk, dtype=np.float32)
    wv = np.asarray(wv, dtype=np.float32)
    wo = np.asarray(wo, dtype=np.float32)

    # x pre-tiled: [P, global token tile, d-chunk, token] with contiguous
    # per-partition runs per (tile, d-chunk)
    xT = x.reshape(B * S, D).T                        # [D, B*S]
    xTt = np.ascontiguousarray(
        xT.reshape(ND, P, B * NTT, TT).transpose(1, 2, 0, 3)).astype(bf)

    def tile_w(w):  # [D, FPC] -> [P, ND, FPC]
        return np.ascontiguousarray(
            w.reshape(ND, P, FPC).transpose(1, 0, 2)).astype(bf)

    # permute q/k head dims: per head, even dims then odd dims
    perm = np.concatenate(
        [h * HD + np.concatenate([np.arange(0, HD, 2), np.arange(1, HD, 2)])
         for h in range(H)]
    )
    wq_p = wq[perm]
    wk_p = wk[perm]

    # rope tables; cos stacked twice, sin stacked [s; -s]; replicated over
    # the 2-head pair dim for the paired evictions
    inv_freq = 1.0 / (10000.0 ** (np.arange(0, HD, 2, dtype=np.float64) / HD))
    t = np.arange(S, dtype=np.float64)
    freqs = t[:, None] * inv_freq[None, :]            # [S, 64]
    cosT = np.cos(freqs).T.astype(np.float32)         # [64, S]
    sinT = np.sin(freqs).T.astype(np.float32)
    cos1 = np.vstack([cosT, cosT])                    # [128, S]
    sin1 = np.vstack([sinT, -sinT])
    cosS = np.ascontiguousarray(
        np.broadcast_to(cos1[:, None, :], (P, 2, S))).astype(bf)
    sinS = np.ascontiguousarray(
        np.broadcast_to(sin1[:, None, :], (P, 2, S))).astype(bf)

    # masks: [zeros(128) | lower-triangular(128)] for the diagonal blocks
    pidx = np.arange(P)[:, None]
    qidx = np.arange(P)[None, :]
    tri = (qidx >= pidx).astype(np.float32)
    m = np.ascontiguousarray(
        np.hstack([np.zeros((P, P), np.float32), tri])).astype(bf)
    ones = np.ones((P, P), dtype=np.float32).astype(bf)

    in_maps = []
    for c in range(NCORES):
        fs = slice(c * FPC, (c + 1) * FPC)
        woc = wo[:, fs].T                              # [256, D]
        in_maps.append({
            "xTb": xTt,
            "wqT": tile_w(wq_p[fs].T),                 # [P, ND, FPC]
            "wkT": tile_w(wk_p[fs].T),
            "wvT": tile_w(wv[fs].T),
            "woT": np.ascontiguousarray(
                woc.reshape(2, P, D).transpose(1, 0, 2)).astype(bf),
            "cosS": cosS,
            "sinS": sinS,
            "masks": m,
            "onesd": ones,
        })
    return in_maps


def _run(inputs, trace=False):
    from concourse.bass_utils import run_bass_kernel_spmd

    if "nc" not in _CACHE:
        _CACHE["nc"] = _build_nc()
    nc = _CACHE["nc"]

    in_maps = _host_prep(
        inputs["x"], inputs["wq"], inputs["wk"], inputs["wv"], inputs["wo"]
    )
    res = run_bass_kernel_spmd(nc, in_maps, list(range(NCORES)), trace=trace)
    acc = None
    for c in range(NCORES):
        part = res.results[c]["outp"].astype(np.float32)
        acc = part if acc is None else acc + part
    out = acc.reshape(B, S, D).astype(np.float32)
    return out, res


def kernel(**inputs) -> np.ndarray:
    out, _ = _run(inputs, trace=False)
    return out
